# revision 55
# baseline (speedup 1.0000x reference)
"""FAMoE layer Trainium2 kernel (v5).

Per batch row j of x [B, H, L] (pure data parallel, B/8 rows per core):

  Input DMA (E-layout): h = 512*s + 4*p + q -> x_nat[p, (b, s, q, l)] bf16,
  descriptors read 4 consecutive h-rows (800 B); the first chunk uses
  2-row groups so the pipeline starts ~9 us earlier.

  8 narrow PE transposes ([128, 64]) -> p_xt; ONE pair-wide DVE copy
  -> xt2 [114, 1024] per row-pair (halves sem round trips on the
  PE-feeding path): rows (s, l) at 0-49/64-113, cols (q, p) per half.

  fwd DFT p_cs = ri2^T @ xt; ACT Square -> sq; PE fold matmul packs
  C^2+S^2 of even/odd rows into halves of one [128, 512] psum tile
  (fold stationary zero-pads rows 52:64), so ONE ACT Sqrt+accum serves a
  row-PAIR on full lanes; strided de-pack copies -> gbuf [52, nb].

  Gating MLP per chunk of NB rows (psum scratch shares the mquad ring so
  the fold psum ring is never blocked) -> wrep [116, NB].

  Inverse without a cs copy: out = (ri2c @ lo_j)^T @ xt, quad-batched m;
  msbq f32->bf16 cast on the scalar engine. The PSUM->SBUF out-copy
  permutes (q,p)->h order; output descriptors are 2 KB and the output DMA
  is issued from the gpsimd queue (software DGE) so it spreads across all
  16 DMA engines like the input stream (hwdge only reached 10).

  The main loop interleaves chunk k+1's forward phase with chunk k's
  inverse phase (inverse lags LAG=10 steps so the gating-MLP chain is
  covered by queued fwd work); in the tail chunk, where no fwd work
  remains and DVE paces the inverse, half the out-copies run on ACT.
"""

import sys

sys.path.insert(0, "/opt/trn_rl_repo")

import numpy as np

N_CORES = 8
B, H, L = 2048, 1024, 50
F = 26
E = 8
BS = B // N_CORES          # 256 batch rows per core
NB = 32                    # gating chunk size
GRP = 8                    # batch rows per input DMA group
OJ = 4                     # batch rows per output DMA group
QJ = 4                     # quad-batch for m matmuls

# feature flags (fallbacks if hardware rejects a pattern)
STRIDED_PXT = False        # transposes write h-ordered (stride-4) psum cols
INV_RHS_PERM = False       # inverse matmul reads xt cols in h order instead
MSBQ_ON_ACT = True        # msbq f32->bf16 cast on scalar engine
OUT_ON_SWDGE = True        # output DMA via gpsimd (16-engine spread)
TRANSP_SPLIT2 = False      # 4 transposes/row with 2-segment (s,l) stationary
TRANSP_PAIR = False        # blocked: codegen ties out partitions to the
                           # matmul's own (single) weights AP

_cache = {}


def _dft_consts():
    l = np.arange(L)[:, None].astype(np.float64)
    f = np.arange(F)[None, :].astype(np.float64)
    ang = 2.0 * np.pi * l * f / L
    R = np.cos(ang)
    I = -np.sin(ang)
    c = np.full(F, 2.0)
    c[0] = 1.0
    c[F - 1] = 1.0
    A = c[None, :] * np.cos(ang) / L
    B2 = -c[None, :] * np.sin(ang) / L
    return R, I, A, B2


def _build_ri2(R, I):
    RI2 = np.zeros((114, 128), np.float64)
    RI2[0:50, 0:26] = R
    RI2[64:114, 26:52] = R
    RI2[0:50, 64:90] = I
    RI2[64:114, 90:116] = I
    return RI2


def _build_abb(A, B2):
    ABB = np.zeros((116, 128), np.float64)
    ABB[0:26, 0:50] = A.T
    ABB[26:52, 64:114] = A.T
    ABB[64:90, 0:50] = B2.T
    ABB[90:116, 64:114] = B2.T
    return ABB


def _build_fold():
    Fm = np.zeros((116, 64), np.float32)
    Fm[np.arange(52), np.arange(52)] = 1.0
    Fm[64 + np.arange(52), np.arange(52)] = 1.0
    return Fm


def _build_sel():
    S = np.zeros((26, 116), np.float32)
    for base in (0, 26, 64, 90):
        S[np.arange(26), base + np.arange(26)] = 1.0
    return S


def _build_program(bs, nb):
    from concourse import bacc, bass, mybir, tile

    f32 = mybir.dt.float32
    bf16 = mybir.dt.bfloat16

    nc = bacc.Bacc("TRN2", target_bir_lowering=False, debug=False)

    x_d = nc.dram_tensor("x", [bs, H, L], f32, kind="ExternalInput")
    out_d = nc.dram_tensor("out", [bs, L, H], f32, kind="ExternalOutput")
    ident_d = nc.dram_tensor("ident", [128, 128], bf16, kind="ExternalInput")
    ri2_d = nc.dram_tensor("ri2", [114, 128], bf16, kind="ExternalInput")
    ri2ct_d = nc.dram_tensor("ri2ct", [116, 128], bf16, kind="ExternalInput")
    abb_d = nc.dram_tensor("abb", [116, 128], bf16, kind="ExternalInput")
    fold_d = nc.dram_tensor("fold", [116, 64], bf16, kind="ExternalInput")
    sel_d = nc.dram_tensor("sel", [F, 116], f32, kind="ExternalInput")
    w1f_d = nc.dram_tensor("w1f", [2 * F, F], f32, kind="ExternalInput")
    b1_d = nc.dram_tensor("b1c", [F, 1], f32, kind="ExternalInput")
    w2_d = nc.dram_tensor("w2", [F, E], f32, kind="ExternalInput")
    b2_d = nc.dram_tensor("b2c", [E, 1], f32, kind="ExternalInput")
    mask_d = nc.dram_tensor("mask", [E, F], f32, kind="ExternalInput")
    ones8_d = nc.dram_tensor("ones8", [E, 1], f32, kind="ExternalInput")
    ones8r_d = nc.dram_tensor("ones8r", [1, E], f32, kind="ExternalInput")

    n_chunk = bs // nb
    ngrp = nb // GRP

    with tile.TileContext(nc) as tc:
        with (
            tc.tile_pool(name="consts", bufs=1) as cpool,
            tc.tile_pool(name="xin", bufs=2 * ngrp + 1) as xpool,
            tc.tile_pool(name="xt", bufs=nb + 2) as xtpool,
            tc.tile_pool(name="sq", bufs=4) as sqpool,
            tc.tile_pool(name="mag", bufs=3) as magpool,
            tc.tile_pool(name="mw", bufs=3) as mwpool,
            tc.tile_pool(name="gat", bufs=2) as gpool,
            tc.tile_pool(name="osb", bufs=3) as opool,
            tc.tile_pool(name="ps_xt", bufs=2, space="PSUM") as ps_xt,
            tc.tile_pool(name="ps_cs", bufs=2, space="PSUM") as ps_cs,
            tc.tile_pool(name="ps_m", bufs=1, space="PSUM") as ps_m,
            tc.tile_pool(name="ps_f", bufs=1, space="PSUM") as ps_f,
            tc.tile_pool(name="ps_o", bufs=2, space="PSUM") as ps_o,
        ):
            ident = cpool.tile([128, 128], bf16)
            ri2 = cpool.tile([114, 128], bf16)
            ri2ct = cpool.tile([116, 128], bf16)
            abb = cpool.tile([116, 128], bf16)
            fold = cpool.tile([116, 64], bf16)
            sel = cpool.tile([F, 116], f32)
            w1f = cpool.tile([2 * F, F], f32)
            b1 = cpool.tile([F, 1], f32)
            w2 = cpool.tile([F, E], f32)
            b2 = cpool.tile([E, 1], f32)
            mask = cpool.tile([E, F], f32)
            ones8 = cpool.tile([E, 1], f32)
            ones8r = cpool.tile([1, E], f32)
            for t, d in [
                (ident, ident_d), (ri2, ri2_d), (ri2ct, ri2ct_d),
                (abb, abb_d), (fold, fold_d), (sel, sel_d), (w1f, w1f_d),
                (b1, b1_d), (w2, w2_d), (b2, b2_d), (mask, mask_d),
                (ones8, ones8_d), (ones8r, ones8r_d),
            ]:
                nc.sync.dma_start(t[:], d[:])

            Sq = mybir.ActivationFunctionType.Square
            Sqrt = mybir.ActivationFunctionType.Sqrt
            Copy = mybir.ActivationFunctionType.Copy
            Relu = mybir.ActivationFunctionType.Relu
            Exp = mybir.ActivationFunctionType.Exp
            MUL = mybir.AluOpType.mult
            ADD = mybir.AluOpType.add

            XPAD = 200 if TRANSP_SPLIT2 else 64
            LAG = 10           # inverse emission lag behind fwd steps
            GRP0 = 2           # finer first-chunk groups to cut startup stall

            def emit_group_dma(bb, nb_rows):
                x_nat = xpool.tile(
                    [128, GRP * 400 + XPAD], bf16, tag="xnat"
                )
                src = x_d[bb : bb + nb_rows].rearrange(
                    "b (s p q) l -> p b s (q l)", s=2, p=128, q=4
                )
                dst = x_nat[:, 0 : nb_rows * 400].rearrange(
                    "p (b s ql) -> p b s ql", b=nb_rows, s=2, ql=200
                )
                nc.gpsimd.dma_start(out=dst, in_=src)
                nc.gpsimd.memset(
                    x_nat[:, nb_rows * 400 : nb_rows * 400 + XPAD], 0.0
                )
                return x_nat

            # per-chunk state: x_nat group tiles, xt tiles, gbuf, wrep
            st = {}

            def _mm_noload(out, lhsT_dummy, rhs):
                # InstMatmult that consumes the currently loaded PE array
                # (two prior explicit ldweights subtiles) without its own
                # weight load. lhsT_dummy only feeds shape/dep tracking.
                ifmap_ap = nc.tensor.lower_ap(rhs.opt({0}), opt=False)
                weights_ap = nc.tensor.lower_ap(
                    lhsT_dummy.opt({0}), opt=False,
                    for_matmul_weights=True,
                )
                out_ap = nc.tensor.lower_ap(out)
                return nc.tensor.add_instruction(
                    mybir.InstMatmult(
                        name=nc.tensor.bass.get_next_instruction_name(),
                        replication_resolution=0,
                        replication_shift_amnt=0,
                        replication_num_rows=0,
                        start_tensor_calc=True,
                        stop_tensor_calc=True,
                        ins=[ifmap_ap, weights_ap],
                        outs=[out_ap],
                        is_transpose=True,
                        tile_position=(0, 0),
                        tile_size=(128, 64),
                    )
                )

            def fwd_transp(c, j):
                s_ = st[c]
                grp = s_["grp"]
                x_nat = s_["x_nats"][j // grp]
                jl = j % grp
                jh0 = j % 2
                if jh0 == 0:
                    s_["pxt2"] = ps_xt.tile(
                        [128, 1024], bf16, tag="pxt", name="pxt2"
                    )
                p_xt = s_["pxt2"]
                if TRANSP_PAIR:
                    for q in range(4):
                        offA = jl * 400 + q * 50
                        offB = offA + 200
                        nc.tensor.ldweights(
                            x_nat[:, offB : offB + 64],
                            is_transpose=True,
                            tile_position=(0, 64),
                        )
                        _mm_noload(
                            p_xt[0:128,
                                 512 * jh0 + 128 * q : 512 * jh0 + 128 * q + 128],
                            x_nat[:, offA : offA + 64],
                            ident[:],
                        )
                elif TRANSP_SPLIT2:
                    for q in range(4):
                        base = jl * 400 + q * 50
                        lhsT = x_nat[:, base : base + 400].rearrange(
                            "p (s ql) -> p s ql", s=2, ql=200
                        )[:, :, 0:64]
                        nc.tensor.matmul(
                            p_xt[0:128,
                                 512 * jh0 + 128 * q : 512 * jh0 + 128 * q + 128],
                            lhsT,
                            ident[:],
                            is_transpose=True,
                        )
                else:
                    for s in range(2):
                        base = p_xt[64 * s : 64 * s + 64,
                                     512 * jh0 : 512 * jh0 + 512]
                        for q in range(4):
                            off = jl * 400 + s * 200 + q * 50
                            dst = base[:, 128 * q : 128 * q + 128]
                            nc.tensor.matmul(
                                dst,
                                x_nat[:, off : off + 64],
                                ident[:],
                                is_transpose=True,
                            )
                if jh0 == 1:
                    # one pair-wide copy: halves DVE instruction count and
                    # cross-engine sem round trips on the PE-feeding path
                    xt2 = xtpool.tile([114, 1024], bf16, tag="xt", name="xt2")
                    nc.vector.tensor_copy(xt2[:], p_xt[0:114, :])
                    s_["xts"].append(xt2[:, 0:512])
                    s_["xts"].append(xt2[:, 512:1024])

            def fwd_dft(c, j):
                s_ = st[c]
                p_cs = ps_cs.tile([128, 512], f32, tag="pcs", name="pcs")
                nc.tensor.matmul(p_cs[:], ri2[:], s_["xts"][j])
                s_["pcs"][j] = p_cs

            def fwd_square(c, j):
                s_ = st[c]
                sq = sqpool.tile([116, 512], bf16, tag="sq", name="sq")
                nc.scalar.activation(sq[:], s_["pcs"][j][0:116, :], Sq)
                del s_["pcs"][j]
                s_["sq"][j] = sq

            def fwd_fold(c, j):
                # pack even row's C^2+S^2 at psum rows 0:64, odd at 64:128
                # (fold64 stationary zero-pads rows 52:64) so ONE Sqrt+accum
                # serves two batch rows on full ACT lanes.
                s_ = st[c]
                if j % 2 == 0:
                    s_["pfold2"] = ps_f.tile(
                        [128, 512], f32, tag="sm", name="pfold2"
                    )
                pf2 = s_["pfold2"]
                po = 0 if j % 2 == 0 else 64
                nc.tensor.matmul(
                    pf2[po : po + 64, :], fold[:], s_["sq"][j][:]
                )
                del s_["sq"][j]
                if j % 2 == 1:
                    mag = magpool.tile([116, 512], bf16, tag="mag")
                    nc.scalar.activation(
                        mag[:], pf2[0:116, :], Sqrt,
                        accum_out=s_["gbuf2"][:, j // 2 : j // 2 + 1],
                    )

            def emit_depack(c):
                s_ = st[c]
                gbuf = gpool.tile([52, nb], f32, tag="gbuf", name="gbuf")
                gv = gbuf[:].rearrange("f (i two) -> f two i", two=2)
                nc.vector.tensor_copy(gv[:, 0], s_["gbuf2"][0:52, :])
                nc.vector.tensor_copy(gv[:, 1], s_["gbuf2"][64:116, :])
                s_["gbuf"] = gbuf

            FLAG = 6  # total forward-pipeline depth (drain steps)

            def fwd_step(c, t):
                if t < nb:
                    fwd_transp(c, t)
                if 2 <= t <= nb + 1:
                    fwd_dft(c, t - 2)
                if 4 <= t <= nb + 3:
                    fwd_square(c, t - 4)
                if 6 <= t <= nb + 5:
                    fwd_fold(c, t - 6)
                    if t == nb + 5:
                        emit_depack(c)

            def emit_mlp(c):
                s_ = st[c]
                gbuf = s_["gbuf"]
                p_h1 = ps_m.tile([F, nb], f32, tag="pm")
                nc.tensor.matmul(p_h1[:], w1f[:], gbuf[:])
                h1 = gpool.tile([F, nb], f32, tag="h1")
                nc.scalar.activation(h1[:], p_h1[:], Relu, bias=b1[:])
                p_z = ps_m.tile([E, nb], f32, tag="pm")
                nc.tensor.matmul(p_z[:], w2[:], h1[:])
                ez = gpool.tile([E, nb], f32, tag="ez")
                nc.scalar.activation(ez[:], p_z[:], Exp, bias=b2[:])
                p_s = ps_m.tile([1, nb], f32, tag="pm")
                nc.tensor.matmul(p_s[:], ones8[:], ez[:])
                rs = gpool.tile([1, nb], f32, tag="rs")
                nc.vector.reciprocal(rs[:], p_s[:])
                p_r8 = ps_m.tile([E, nb], f32, tag="pm")
                nc.tensor.matmul(p_r8[:], ones8r[:], rs[:])
                ezn = gpool.tile([E, nb], f32, tag="ezn")
                nc.vector.tensor_tensor(ezn[:], ez[:], p_r8[:], MUL)
                p_w = ps_m.tile([F, nb], f32, tag="pm")
                nc.tensor.matmul(p_w[:], mask[:], ezn[:])
                w_sb = gpool.tile([F, nb], f32, tag="wsb")
                nc.vector.tensor_copy(w_sb[:], p_w[:])
                p_wrep = ps_m.tile([116, nb], f32, tag="pm")
                nc.tensor.matmul(p_wrep[:], sel[:], w_sb[:])
                wrep = gpool.tile([116, nb], f32, tag="wrep")
                nc.vector.tensor_copy(wrep[:], p_wrep[:])
                s_["wrep"] = wrep

            def emit_mquad(s_, g, tag="msb", bufs=None):
                if bufs is not None:
                    # tail prebuild: borrow the ps_o ring (bufs=2) so the
                    # mquad->cast chain pipelines instead of serializing
                    # through the single ps_m bank
                    p_mq = ps_o.tile([128, QJ * 128], f32, tag="po")
                else:
                    p_mq = ps_m.tile([128, QJ * 128], f32, tag="pm")
                nc.tensor.matmul(p_mq[:], ri2ct[:], s_["loq"][:])
                msbq = mwpool.tile(
                    [114, QJ * 128], bf16, tag=tag, bufs=bufs, name="msbq"
                )
                if MSBQ_ON_ACT:
                    nc.scalar.activation(msbq[:], p_mq[0:114, :], Copy)
                else:
                    nc.vector.tensor_copy(msbq[:], p_mq[0:114, :])
                s_["msbq"][g] = msbq

            def emit_inv_j(c, j):
                s_ = st[c]
                jq = j % QJ
                g = j // QJ
                wrep = s_["wrep"]
                # build lo / m / msb for quad g+1 while running quad g
                # (the tail chunk prebuilds everything in its prologue)
                jn = j + QJ
                if jn < nb and not s_.get("prebuilt"):
                    if jq == 0:
                        s_["loq"] = mwpool.tile(
                            [116, QJ * 128], bf16, tag="lo", name="loq"
                        )
                    nc.vector.tensor_scalar(
                        s_["loq"][:, jq * 128 : jq * 128 + 128], abb[:],
                        wrep[:, jn : jn + 1], None, MUL,
                    )
                    if jq == QJ - 1:
                        emit_mquad(s_, g + 1)
                # inverse + out-copy for row j (msbq computed a quad ahead)
                msbq = s_["msbq"][g]
                if jq == 0:
                    s_["osb"] = opool.tile(
                        [114, OJ * 512], f32, tag="osb", name="osb"
                    )
                osb = s_["osb"]
                p_o = ps_o.tile([128, 512], f32, tag="po")
                nc.tensor.matmul(
                    p_o[:], msbq[:, jq * 128 : jq * 128 + 128], s_["xts"][j]
                )
                dstv = osb[:, 512 * jq : 512 * jq + 512]
                srcv = p_o[0:114, :].rearrange(
                    "r (q p) -> r p q", q=4, p=128
                )
                if jq % 2 == 1:
                    # ACT has headroom below PE; alternating the out-copies
                    # keeps DVE from backing up the ps_o/ps_xt rings (which
                    # stalls PE at chunk boundaries and paces the tail)
                    nc.scalar.activation(dstv, srcv, Copy)
                else:
                    nc.vector.tensor_copy(dstv, srcv)
                if jq == QJ - 1:
                    b0 = c * nb + j - OJ + 1
                    dma_q = nc.gpsimd if OUT_ON_SWDGE else nc.sync
                    if c == n_chunk - 1 and (j // QJ) % 2 == 1:
                        # tail drain: split issue across the idle sync queue
                        # so swdge generation doesn't serialize the last
                        # groups behind one engine's ucode
                        dma_q = nc.sync
                    dma_q.dma_start(
                        out=out_d[b0 : b0 + OJ, :, 0:512].rearrange(
                            "b l n -> l b n"
                        ),
                        in_=osb[0:50, :].rearrange(
                            "l (b n) -> l b n", b=OJ, n=512
                        ),
                    )
                    dma_q.dma_start(
                        out=out_d[b0 : b0 + OJ, :, 512:1024].rearrange(
                            "b l n -> l b n"
                        ),
                        in_=osb[64:114, :].rearrange(
                            "l (b n) -> l b n", b=OJ, n=512
                        ),
                    )

            def emit_inv_prologue(c):
                # lo / m / msb for quad 0 of chunk c; the tail chunk has no
                # fwd work to hide per-quad DVE->PE->ACT round trips, so
                # prebuild every quad upfront into deep rings.
                s_ = st[c]
                wrep = s_["wrep"]
                s_["msbq"] = {}
                quads = 1  # tail prebuild measured neutral; keep streaming
                if quads > 1:
                    s_["prebuilt"] = True
                for g0 in range(quads):
                    s_["loq"] = mwpool.tile(
                        [116, QJ * 128], bf16,
                        tag="lo8" if quads > 1 else "lo",
                        bufs=2 if quads > 1 else None, name="loq"
                    )
                    for i in range(QJ):
                        nc.vector.tensor_scalar(
                            s_["loq"][:, i * 128 : i * 128 + 128], abb[:],
                            wrep[:, g0 * QJ + i : g0 * QJ + i + 1], None, MUL,
                        )
                    emit_mquad(
                        s_, g0,
                        tag="msb8" if quads > 1 else "msb",
                        bufs=nb // QJ if quads > 1 else None,
                    )

            def new_chunk_state(c, grp=GRP):
                st[c] = {
                    "x_nats": None,
                    "xts": [],
                    "pcs": {},
                    "sq": {},
                    "grp": grp,
                    "gbuf2": gpool.tile(
                        [116, nb // 2], f32, tag="gbuf2", name="gbuf2"
                    ),
                }

            # ---------------- prologue ----------------
            new_chunk_state(0, grp=GRP0)
            st[0]["x_nats"] = [
                emit_group_dma(g * GRP0, GRP0) for g in range(nb // GRP0)
            ]
            if n_chunk > 1:
                new_chunk_state(1)
                st[1]["x_nats"] = [
                    emit_group_dma(nb + g * GRP, GRP) for g in range(ngrp)
                ]
            for t in range(nb + 6):
                fwd_step(0, t)

            # ---------------- main: fwd(k+1) leads, then inv(k) ----------
            for k in range(n_chunk):
                if k + 2 < n_chunk:
                    new_chunk_state(k + 2)
                    st[k + 2]["x_nats"] = []
                for t in range(nb + LAG):
                    # spread input-group issues through the loop so the
                    # pool FIFO never blocks ready output DMAs behind a
                    # burst of input descriptor-generation
                    if k + 2 < n_chunk and t % 8 == 1 and t // 8 < ngrp:
                        st[k + 2]["x_nats"].append(
                            emit_group_dma((k + 2) * nb + (t // 8) * GRP, GRP)
                        )
                    if k + 1 < n_chunk:
                        fwd_step(k + 1, t)
                    if t == 5:
                        emit_mlp(k)
                        emit_inv_prologue(k)
                    if LAG <= t:
                        emit_inv_j(k, t - LAG)
                del st[k]

    nc.compile()
    return nc


def _get_program(bs=BS, nb=NB):
    key = (bs, nb)
    if key not in _cache:
        _cache[key] = _build_program(bs, nb)
    return _cache[key]


def _host_consts(band_boundaries, W1, b1, W2, b2):
    import ml_dtypes

    bf = ml_dtypes.bfloat16
    R, I, A, B2 = _dft_consts()
    sig = 1.0 / (1.0 + np.exp(-band_boundaries.astype(np.float64)))
    bounds = np.concatenate([[0.0], np.sort(sig), [1.0]])
    idx = (bounds * F).astype(np.int32)
    idx[-1] = F
    k = np.arange(F)
    mask = (
        (k[None, :] >= idx[:-1, None]) & (k[None, :] < idx[1:, None])
    ).astype(np.float32)
    ri2 = _build_ri2(R, I)
    w1f = np.concatenate([W1, W1], axis=0).astype(np.float64) * (1.0 / 1024.0)
    return {
        "ident": np.eye(128, dtype=np.float32).astype(bf),
        "ri2": ri2.astype(np.float32).astype(bf),
        "ri2ct": np.pad(ri2[:, 0:116].T, ((0, 0), (0, 14))).astype(np.float32).astype(bf),
        "abb": _build_abb(A, B2).astype(np.float32).astype(bf),
        "fold": _build_fold().astype(bf),
        "sel": _build_sel(),
        "w1f": w1f.astype(np.float32),
        "b1c": b1.reshape(F, 1).astype(np.float32),
        "w2": W2.astype(np.float32),
        "b2c": b2.reshape(E, 1).astype(np.float32),
        "mask": mask,
        "ones8": np.ones((E, 1), np.float32),
        "ones8r": np.ones((1, E), np.float32),
    }


def kernel(x, band_boundaries, W1, b1, W2, b2):
    from concourse.bass_utils import run_bass_kernel_spmd

    nc = _get_program()
    consts = _host_consts(
        np.asarray(band_boundaries), np.asarray(W1), np.asarray(b1),
        np.asarray(W2), np.asarray(b2),
    )
    x = np.ascontiguousarray(np.asarray(x, dtype=np.float32))
    in_maps = [
        {"x": x[i * BS : (i + 1) * BS], **consts} for i in range(N_CORES)
    ]
    res = run_bass_kernel_spmd(nc, in_maps, list(range(N_CORES)))
    return np.concatenate([res.results[i]["out"] for i in range(N_CORES)], axis=0)
